# revision 35
# baseline (speedup 1.0000x reference)
"""ChildSumTreeLSTM with relation transforms on 8 Trainium2 NeuronCores.

Layout: everything transposed (features on SBUF partitions, tree nodes on the
free dim), node columns in topological-wave order (= heap order for the
reference tree). Per wave of parents:
  hsum (DVE strided reduce over child cols) -> rel-sharded PE streams of
  host-pretransposed W blocks (bf16, one stream per (wave, rel-slot)) into
  per-slot PSUM -> per-core 0/1 column masks (input data) zero wrong-rel
  columns -> dense bf16 AllGather + on-chip rank-sum gives every core the full
  ch_sum -> column-sharded iou/f gates (each core owns one 128-feature slice)
  -> small AllGather of the new h columns.
All per-core differences are input data (weights shards, masks, bias slices),
so one Bass program runs SPMD on all 8 cores.
"""

import sys

sys.path.insert(0, "/opt/trn_rl_repo")

import numpy as np
import ml_dtypes

import concourse.bass as bass
import concourse.mybir as mybir
import concourse.tile as tile
from concourse.bass_utils import run_bass_kernel_spmd
from concourse.vector_clock import ScopedClock, VectorClock

BF16 = mybir.dt.bfloat16
F32 = mybir.dt.float32
NCORES = 8
P = 128

# This walrus build rejects >1 sem wait per instruction at the Tile exit
# drain; split the aggregated drain into one drain per proc.
def _split_drain_and_barrier(self, tick_clock, wait_clock):
    gc = tick_clock.global_clock
    n = len(gc)
    nonzero = [i for i in range(n) if gc[i] > 0]
    for j in nonzero:
        vec = VectorClock([gc[i] if i == j else 0 for i in range(n)])
        d = self.nc.sync.drain()
        wait_clock.add_sem_waits(d.ins, ScopedClock({None: vec}))
    if not nonzero:
        d = self.nc.sync.drain()
        wait_clock.add_sem_waits(d.ins, ScopedClock({None: gc.copy()}))
    self.nc.all_engine_barrier()
    assert self.sems is not None
    popped = self.nc._tile_sem_poison_stack.pop()
    assert popped is self._sem_poison
    self.nc.clear_and_free_semaphores(list(self.sems.allocated().values()))
    self.nc.all_engine_barrier()


tile.TileContext._drain_and_barrier = _split_drain_and_barrier


def _split_multi_waits(nc, limit=1):
    """Walrus here allows only one sem wait per instruction; hoist extras
    onto same-engine NOPs inserted right before the instruction."""
    for bb in nc.main_func.blocks:
        new_list = []
        for ins in bb.instructions:
            si = getattr(ins, "sync_info", None)
            if si is not None and si.on_wait and len(si.on_wait) > limit:
                waits = list(si.on_wait)
                for w in waits[:-limit]:
                    nop = mybir.InstNoOp(
                        name=nc.get_next_instruction_name(),
                        sync_info=mybir.SyncInfo(on_wait=[w], on_update=[]),
                        bass_nofuse=True,
                        engine=ins.engine,
                    )
                    nc.register_instruction(nop, overwrite=True)
                    new_list.append(nop)
                si.on_wait = waits[-limit:]
            new_list.append(ins)
        bb.instructions[:] = new_list


def _bf16(a):
    return np.ascontiguousarray(a.astype(ml_dtypes.bfloat16))


def _blocksT(mat):
    """[M, K] -> [K/128 * M/128, 128, 128] of transposed blocks, k-major order
    grouped as [m, k] -> index m*KC + k, each block = mat[mb, kb].T (lhsT)."""
    M, K = mat.shape
    MC, KC = M // P, K // P
    out = np.empty((MC * KC, P, P), mat.dtype)
    for m in range(MC):
        for k in range(KC):
            out[m * KC + k] = mat[m * P:(m + 1) * P, k * P:(k + 1) * P].T
    return out


def _plan(child_idx, rel_ids, Wrel):
    """Host-side planning: waves, column order, rel->core assignment, slots."""
    N, K = child_idx.shape
    R1 = Wrel.shape[0]
    eff_children = []
    wave = np.zeros(N, np.int32)
    for i in range(N):
        cs = [int(c) for c in child_idx[i] if 0 <= c < i]
        eff_children.append(cs)
        wave[i] = 1 + max((wave[c] for c in cs), default=-1)
    nwaves = int(wave.max()) + 1
    # column order: by (wave, descending node) -> for the reference heap tree
    # this is exactly heap order (col j = node N-1-j) keeping children of
    # consecutive parents contiguous.
    order = sorted(range(N), key=lambda i: (wave[i], -i))
    col_of = np.empty(N, np.int64)
    for j, node in enumerate(order):
        col_of[node] = j
    waves = []  # list of (p0, p1) col ranges
    j = 0
    for w in range(nwaves):
        cnt = int((wave == w).sum())
        waves.append((j, j + cnt))
        j += cnt

    ident = set()
    eye = np.eye(Wrel.shape[1], dtype=Wrel.dtype)
    for r in set(int(rel_ids[i]) for i in range(N)):
        if np.array_equal(Wrel[r], eye):
            ident.add(r)

    # per wave (>=1): rels present; identity rels are skipped only when the
    # whole wave is identity (then ch_sum == hsum, no matmul or exchange)
    wave_rels = []
    for w in range(1, nwaves):
        p0, p1 = waves[w]
        rels_all = set(int(rel_ids[order[j]]) for j in range(p0, p1))
        if rels_all <= ident:
            wave_rels.append([])
        else:
            wave_rels.append(sorted(rels_all))

    # static rel->core assignment, greedy balance on total appearances
    from collections import defaultdict
    count = defaultdict(int)
    for rels in wave_rels:
        for r in rels:
            count[r] += 1
    nw = len(wave_rels)
    loadw = [[0] * nw for _ in range(NCORES)]
    assign = {}
    for r in sorted(count, key=lambda r: -count[r]):
        pres = [wi for wi in range(nw) if r in wave_rels[wi]]
        best, bkey = 0, None
        for c in range(NCORES):
            key = (sum(loadw[c][wi] for wi in pres), sum(loadw[c]))
            if bkey is None or key < bkey:
                best, bkey = c, key
        assign[r] = best
        for wi in pres:
            loadw[best][wi] += 1

    # per wave: per-core slot lists, padded to n_s
    wave_slots = []  # per internal wave: (n_s, slots[c] lists possibly short)
    for rels in wave_rels:
        per_core = [[r for r in rels if assign[r] == c] for c in range(NCORES)]
        n_s = max((len(x) for x in per_core), default=0)
        wave_slots.append((n_s, per_core))
    return dict(order=order, col_of=col_of, waves=waves, wave=wave,
                eff_children=eff_children, ident=ident,
                wave_slots=wave_slots, nwaves=nwaves)


def kernel(**inputs):
    x = np.asarray(inputs["x"], np.float32)
    Wrel = np.asarray(inputs["Wrel"], np.float32)
    ioux_w = np.asarray(inputs["ioux_w"], np.float32)
    ioux_b = np.asarray(inputs["ioux_b"], np.float32)
    iouh_w = np.asarray(inputs["iouh_w"], np.float32)
    iouh_b = np.asarray(inputs["iouh_b"], np.float32)
    fx_w = np.asarray(inputs["fx_w"], np.float32)
    fx_b = np.asarray(inputs["fx_b"], np.float32)
    fh_w = np.asarray(inputs["fh_w"], np.float32)
    fh_b = np.asarray(inputs["fh_b"], np.float32)
    child_idx = np.asarray(inputs["child_idx"], np.int32)
    rel_ids = np.asarray(inputs["rel_ids"], np.int32)

    N, IN_DIM = x.shape
    MEM = fh_w.shape[0]
    KC = MEM // P           # 8 feature chunks
    KX = IN_DIM // P        # 8 input chunks
    K = child_idx.shape[1]  # max children (4)
    NPAD = N + K + 4

    plan = _plan(child_idx, rel_ids, Wrel)
    order, col_of, waves = plan["order"], plan["col_of"], plan["waves"]
    eff_children, ident = plan["eff_children"], plan["ident"]
    wave_slots, nwaves = plan["wave_slots"], plan["nwaves"]

    # Child gather plan: for each internal wave, the flattened (parent-major)
    # child column sequence, decomposed into maximal +1-contiguous runs.
    # Missing children point at the zero pad column ZCOL.
    ZCOL = N
    child_col = np.full((N, K), ZCOL, np.int64)
    for i in range(N):
        for kk, c in enumerate(eff_children[i]):
            child_col[i, kk] = col_of[c]
    wave_runs = []  # per internal wave: list of (dst_off, src_col, length)
    for w in range(1, nwaves):
        p0, p1 = waves[w]
        seq = []
        for j in range(p0, p1):
            seq.extend(child_col[order[j]])
        runs = []
        i0 = 0
        while i0 < len(seq):
            i1 = i0 + 1
            while i1 < len(seq) and seq[i1] == seq[i1 - 1] + 1:
                i1 += 1
            runs.append((i0, int(seq[i0]), i1 - i0))
            i0 = i1
        wave_runs.append(runs)

    # ---- per-core host data -------------------------------------------------
    xT = np.ascontiguousarray(x[order].T)  # [IN_DIM, N] in column order
    xT_b = np.zeros((KX, P, N), ml_dtypes.bfloat16)
    for k in range(KX):
        xT_b[k] = _bf16(xT[k * P:(k + 1) * P])

    S_total = sum(ns for ns, _ in wave_slots)
    MC = MEM // P
    wstream = [np.zeros((max(S_total, 1), P, MC * KC, P), ml_dtypes.bfloat16)
               for _ in range(NCORES)]
    NMAX = max((waves[w][1] - waves[w][0]) for w in range(1, nwaves)) if nwaves > 1 else 1
    NBIG = max(p1 - p0 for p0, p1 in waves)
    PSN = 128  # psum column pad so each m-chunk slice stays inside one bank
    assert NMAX <= PSN and K * NMAX <= 512
    masks = [np.zeros((max(S_total, 1), KC, NMAX), np.float32) for _ in range(NCORES)]
    soff = 0
    for wi, (ns, per_core) in enumerate(wave_slots):
        w = wi + 1
        p0, p1 = waves[w]
        n = p1 - p0
        for c in range(NCORES):
            for s, r in enumerate(per_core[c]):
                wstream[c][soff + s] = _blocksT(Wrel[r]).transpose(1, 0, 2)
                for t in range(n):
                    if int(rel_ids[order[p0 + t]]) == r:
                        masks[c][soff + s, :, t] = 1.0
        soff += ns

    iouxstat = [np.zeros((KX * 3, P, P), ml_dtypes.bfloat16) for _ in range(NCORES)]
    iouhstat = [np.zeros((KC * 3, P, P), ml_dtypes.bfloat16) for _ in range(NCORES)]
    fxstat = [np.zeros((KX, P, P), ml_dtypes.bfloat16) for _ in range(NCORES)]
    fhstat = [np.zeros((KC, P, P), ml_dtypes.bfloat16) for _ in range(NCORES)]
    b_xi = [np.zeros((3, P), np.float32) for _ in range(NCORES)]
    b_iou = [np.zeros((3, P), np.float32) for _ in range(NCORES)]
    b_xf = [np.zeros((P,), np.float32) for _ in range(NCORES)]
    b_fh = [np.zeros((P,), np.float32) for _ in range(NCORES)]
    for c in range(NCORES):
        rows = slice(c * P, (c + 1) * P)
        for g in range(3):
            gr = slice(g * MEM + c * P, g * MEM + (c + 1) * P)
            b_xi[c][g] = ioux_b[gr]
            b_iou[c][g] = iouh_b[gr]
            for k in range(KX):
                iouxstat[c][k * 3 + g] = _bf16(
                    ioux_w[gr, k * P:(k + 1) * P].T)
            for k in range(KC):
                iouhstat[c][k * 3 + g] = _bf16(
                    iouh_w[gr, k * P:(k + 1) * P].T)
        b_xf[c] = fx_b[rows]
        b_fh[c] = fh_b[rows]
        for k in range(KX):
            fxstat[c][k] = _bf16(fx_w[rows, k * P:(k + 1) * P].T)
        for k in range(KC):
            fhstat[c][k] = _bf16(fh_w[rows, k * P:(k + 1) * P].T)

    # ---- build program ------------------------------------------------------
    nc = bass.Bass("TRN2", target_bir_lowering=False, debug=False,
                   num_devices=NCORES)
    d_ws = nc.dram_tensor("wstream", list(wstream[0].shape), BF16,
                          kind="ExternalInput")
    masks_x = [np.ascontiguousarray(
        np.broadcast_to(m[None], (P,) + m.shape)) for m in masks]
    d_mask = nc.dram_tensor("masks", list(masks_x[0].shape), F32,
                            kind="ExternalInput")
    d_xt = nc.dram_tensor("xt", [KX, P, N], BF16, kind="ExternalInput")
    d_iouxs = nc.dram_tensor("iouxstat", [KX * 3, P, P], BF16, kind="ExternalInput")
    d_iouhs = nc.dram_tensor("iouhstat", [KC * 3, P, P], BF16, kind="ExternalInput")
    d_fxs = nc.dram_tensor("fxstat", [KX, P, P], BF16, kind="ExternalInput")
    d_fhs = nc.dram_tensor("fhstat", [KC, P, P], BF16, kind="ExternalInput")
    d_bxi = nc.dram_tensor("b_xi", [3, P], F32, kind="ExternalInput")
    d_biou = nc.dram_tensor("b_iou", [3, P], F32, kind="ExternalInput")
    d_bxf = nc.dram_tensor("b_xf", [P], F32, kind="ExternalInput")
    d_bfh = nc.dram_tensor("b_fh", [P], F32, kind="ExternalInput")
    d_hout = nc.dram_tensor("hout", [P, N], F32, kind="ExternalOutput")

    with tile.TileContext(nc, num_cores=NCORES) as tc:
        with (
            tc.tile_pool(name="const", bufs=1) as cpool,
            tc.tile_pool(name="state", bufs=1) as spool,
            tc.tile_pool(name="wstage", bufs=4) as wpool,
            tc.tile_pool(name="mskp", bufs=4) as mp,
            tc.tile_pool(name="work", bufs=1) as wk,
            tc.tile_pool(name="psum", bufs=2, space="PSUM") as pp,
            tc.tile_pool(name="psg", bufs=2, space="PSUM") as pg,
            tc.tile_pool(name="dram", bufs=2, space="DRAM") as dp,
        ):
            # constants
            xt = cpool.tile([P, KX, N], BF16)
            nc.sync.dma_start(xt[:], d_xt.ap().rearrange("k p n -> p k n"))
            iouxs = cpool.tile([P, KX * 3, P], BF16)
            nc.sync.dma_start(iouxs[:], d_iouxs.ap().rearrange("s p m -> p s m"))
            iouhs = cpool.tile([P, KC * 3, P], BF16)
            nc.sync.dma_start(iouhs[:], d_iouhs.ap().rearrange("s p m -> p s m"))
            fxs = cpool.tile([P, KX, P], BF16)
            nc.sync.dma_start(fxs[:], d_fxs.ap().rearrange("s p m -> p s m"))
            fhs = cpool.tile([P, KC, P], BF16)
            nc.sync.dma_start(fhs[:], d_fhs.ap().rearrange("s p m -> p s m"))
            bxi = cpool.tile([P, 3], F32)
            nc.sync.dma_start(bxi[:], d_bxi.ap().rearrange("g p -> p g"))
            biou = cpool.tile([P, 3], F32)
            nc.sync.dma_start(biou[:], d_biou.ap().rearrange("g p -> p g"))
            bxf = cpool.tile([P, 1], F32)
            nc.sync.dma_start(bxf[:], d_bxf.ap().rearrange("(p one) -> p one", one=1))
            bfh = cpool.tile([P, 1], F32)
            nc.sync.dma_start(bfh[:], d_bfh.ap().rearrange("(p one) -> p one", one=1))

            # state
            h_bf = spool.tile([P, KC, NPAD], BF16)
            nc.vector.memset(h_bf[:], 0.0)
            c_sl = spool.tile([P, NPAD], F32)
            nc.vector.memset(c_sl[:], 0.0)
            h_sl = spool.tile([P, N], F32)
            xi_f = spool.tile([P, 3, N], F32)
            xf_f = spool.tile([P, N], F32)

            # ---- precompute xi/xf (column-sharded: this core's slice) ------
            CCH = PSN
            for cc in range(0, N, CCH):
                ncc = min(CCH, N - cc)
                ps = pg.tile([P, 3, PSN], F32, tag="ps3")
                for g in range(3):
                    for k in range(KX):
                        nc.tensor.matmul(
                            ps[:, g, :ncc],
                            iouxs[:, k * 3 + g, :],
                            xt[:, k, cc:cc + ncc],
                            start=(k == 0), stop=(k == KX - 1))
                for g in range(3):
                    nc.scalar.activation(
                        xi_f[:, g, cc:cc + ncc], ps[:, g, :ncc],
                        mybir.ActivationFunctionType.Identity,
                        bias=bxi[:, g:g + 1])
                psf0 = pg.tile([P, K * NMAX], F32, tag="psf")
                for k in range(KX):
                    nc.tensor.matmul(
                        psf0[:, :ncc], fxs[:, k, :], xt[:, k, cc:cc + ncc],
                        start=(k == 0), stop=(k == KX - 1))
                nc.scalar.activation(
                    xf_f[:, cc:cc + ncc], psf0[:, :ncc],
                    mybir.ActivationFunctionType.Identity, bias=bxf[:, 0:1])

            ACT = mybir.ActivationFunctionType

            def gates(p0, n, iou_ps, fh_src, ccg, nch, big=False):
                """Column-sharded gate math for parents at cols [p0, p0+n).
                iou_ps: [P,3,n] matmul accumulation (xi and biases added
                here). fh_src: [P, nch] fp32 fh contributions for the child
                cols, or None for leaves (no children terms)."""
                NW = NBIG if big else NMAX
                sfx = "L" if big else ""
                tmp = wk.tile([P, 3, NW], F32, tag="gtmp" + sfx)
                nc.vector.tensor_add(tmp[:, :, :n], iou_ps, xi_f[:, :, p0:p0 + n])
                ig = wk.tile([P, NW], F32, tag="ig" + sfx)
                og = wk.tile([P, NW], F32, tag="og" + sfx)
                ug = wk.tile([P, NW], F32, tag="ug" + sfx)
                nc.scalar.activation(ig[:, :n], tmp[:, 0, :n], ACT.Sigmoid,
                                     bias=biou[:, 0:1])
                nc.scalar.activation(og[:, :n], tmp[:, 1, :n], ACT.Sigmoid,
                                     bias=biou[:, 1:2])
                nc.scalar.activation(ug[:, :n], tmp[:, 2, :n], ACT.Tanh,
                                     bias=biou[:, 2:3])
                cn = wk.tile([P, NW], F32, tag="cn" + sfx)
                nc.vector.tensor_mul(cn[:, :n], ig[:, :n], ug[:, :n])
                if fh_src is not None:
                    # f = sigmoid(fh + xf[parent] + b); fc = sum_k f*cc
                    fsb = wk.tile([P, K * NMAX], F32, tag="fsb")
                    xfb = wk.tile([P, K * NMAX], F32, tag="xfb")
                    xfb_v = xfb[:, :nch].rearrange("p (n k) -> p n k", k=K)
                    for kk in range(K):
                        nc.vector.tensor_copy(
                            xfb_v[:, :, kk:kk + 1],
                            xf_f[:, p0:p0 + n].rearrange(
                                "p (n one) -> p n one", one=1))
                    nc.vector.tensor_add(fsb[:, :nch], fh_src, xfb[:, :nch])
                    nc.scalar.activation(fsb[:, :nch], fsb[:, :nch],
                                         ACT.Sigmoid, bias=bfh[:, 0:1])
                    nc.vector.tensor_mul(fsb[:, :nch], fsb[:, :nch],
                                         ccg[:, :nch])
                    fc = wk.tile([P, NMAX], F32, tag="fc")
                    nc.vector.tensor_reduce(
                        fc[:, :n],
                        fsb[:, :nch].rearrange("p (n k) -> p n k", k=K),
                        axis=mybir.AxisListType.X, op=mybir.AluOpType.add)
                    nc.vector.tensor_add(cn[:, :n], cn[:, :n], fc[:, :n])
                nc.vector.tensor_copy(c_sl[:, p0:p0 + n], cn[:, :n])
                tc_t = wk.tile([P, NW], F32, tag="tct" + sfx)
                nc.scalar.activation(tc_t[:, :n], cn[:, :n], ACT.Tanh)
                nc.vector.tensor_mul(h_sl[:, p0:p0 + n], og[:, :n], tc_t[:, :n])

            def publish_h(p0, n, big=False):
                sfx = str(n)
                hb = wk.tile([P, n], BF16, tag="hb" + sfx)
                nc.vector.tensor_copy(hb[:, :n], h_sl[:, p0:p0 + n])
                gin = dp.tile([P, n], BF16, tag="gin" + sfx)
                nc.sync.dma_start(gin[:], hb[:])
                gout = dp.tile([NCORES, P, n], BF16, tag="gout" + sfx,
                               addr_space="Shared")
                nc.gpsimd.collective_compute(
                    "AllGather", mybir.AluOpType.bypass,
                    ins=[gin.opt()], outs=[gout.opt()],
                    replica_groups=[list(range(NCORES))])
                nc.sync.dma_start(
                    h_bf[:, :, p0:p0 + n],
                    gout[:, :, :n].rearrange("k p n -> p k n"))

            # ---- wave 0: leaves -------------------------------------------
            p0, p1 = waves[0]
            n0 = p1 - p0
            iou0 = wk.tile([P, 3, n0], F32, tag="iou0")
            nc.vector.memset(iou0[:], 0.0)
            gates(p0, n0, iou0[:, :, :n0], None, None, 0, big=True)
            publish_h(p0, n0, big=True)

            # ---- internal waves -------------------------------------------
            soff = 0
            for wi in range(1, nwaves):
                ns, per_core = wave_slots[wi - 1]
                p0, p1 = waves[wi]
                n = p1 - p0
                nch = n * K
                # gather child columns (h in bf16, c in fp32) by runs
                hch = wk.tile([P, KC, K * NMAX], BF16, tag="hch")
                ccg = wk.tile([P, K * NMAX], F32, tag="ccg")
                for (dst, src, ln) in wave_runs[wi - 1]:
                    nc.vector.tensor_copy(hch[:, :, dst:dst + ln],
                                          h_bf[:, :, src:src + ln])
                    nc.vector.tensor_copy(ccg[:, dst:dst + ln],
                                          c_sl[:, src:src + ln])
                # hsum over child cols (bf16 in, fp32 out, cast to bf16)
                hsum_f = wk.tile([P, KC, NMAX], F32, tag="hsumf")
                nc.vector.tensor_reduce(
                    hsum_f[:, :, :n],
                    hch[:, :, :nch].rearrange("p k (n c) -> p k n c", c=K),
                    axis=mybir.AxisListType.X, op=mybir.AluOpType.add)
                hsum_b = wk.tile([P, KC, NMAX], BF16, tag="hsumb")
                nc.vector.tensor_copy(hsum_b[:, :, :n], hsum_f[:, :, :n])

                all_id = (ns == 0)
                if not all_id:
                    contrib = wk.tile([P, KC, NMAX], F32, tag="contrib")
                    for s in range(ns):
                        wst = wpool.tile([P, MC * KC, P], BF16, tag="wst")
                        nc.sync.dma_start(wst[:], d_ws.ap()[soff + s])
                        psl = pp.tile([P, MC, PSN], F32, tag="psl")
                        for m in range(MC):
                            for k in range(KC):
                                nc.tensor.matmul(
                                    psl[:, m, :n],
                                    wst[:, m * KC + k, :],
                                    hsum_b[:, k, :n],
                                    start=(k == 0), stop=(k == KC - 1))
                        msk = mp.tile([P, KC, NMAX], F32, tag="msk")
                        nc.sync.dma_start(msk[:, :, :n],
                                          d_mask.ap()[:, soff + s, :, :n])
                        if s == 0:
                            nc.vector.tensor_mul(
                                contrib[:, :, :n], psl[:, :, :n],
                                msk[:, :, :n])
                        else:
                            mt = wk.tile([P, KC, NMAX], F32, tag="mt")
                            nc.vector.tensor_mul(
                                mt[:, :, :n], psl[:, :, :n],
                                msk[:, :, :n])
                            nc.vector.tensor_add(contrib[:, :, :n],
                                                 contrib[:, :, :n],
                                                 mt[:, :, :n])
                    cb = wk.tile([P, KC, n], BF16, tag="cb" + str(n))
                    nc.vector.tensor_copy(cb[:, :, :n], contrib[:, :, :n])
                    # contributions have disjoint support (masked), so the
                    # bf16 CCE adds are exact. Keep each AllReduce payload
                    # <=64KB so the Mesh algorithm is chosen; independent
                    # chunk ARs issued back-to-back can pipeline in ncfw.
                    nsp = 4 if n >= 64 else 1
                    kq = KC // nsp
                    chs_b = wk.tile([P, KC, n], BF16, tag="chsb" + str(n))
                    for q in range(nsp):
                        gi = dp.tile([P, kq, n], BF16,
                                     name="g1i%d_%d" % (q, wi),
                                     tag="g1in%d_%d" % (q, n))
                        nc.sync.dma_start(gi[:], cb[:, q * kq:(q + 1) * kq, :])
                        go = dp.tile([P, kq, n], BF16,
                                     name="g1o%d_%d" % (q, wi),
                                     tag="g1out%d_%d" % (q, n),
                                     addr_space="Shared")
                        nc.gpsimd.collective_compute(
                            "AllReduce", mybir.AluOpType.add,
                            ins=[gi.opt()], outs=[go.opt()],
                            replica_groups=[list(range(NCORES))])
                        nc.sync.dma_start(chs_b[:, q * kq:(q + 1) * kq, :],
                                          go[:])
                    rhs = chs_b
                else:
                    rhs = hsum_b

                # iou matmuls (column-sharded)
                psi = pg.tile([P, 3, PSN], F32, tag="ps3")
                for g in range(3):
                    for k in range(KC):
                        nc.tensor.matmul(
                            psi[:, g, :n], iouhs[:, k * 3 + g, :],
                            rhs[:, k, :n],
                            start=(k == 0), stop=(k == KC - 1))
                # fh matmuls over the gathered child cols
                psf = pg.tile([P, K * NMAX], F32, tag="psf")
                for k in range(KC):
                    nc.tensor.matmul(
                        psf[:, :nch], fhs[:, k, :], hch[:, k, :nch],
                        start=(k == 0), stop=(k == KC - 1))
                gates(p0, n, psi[:, :, :n], psf[:, :nch], ccg, nch)
                if wi < nwaves - 1:
                    publish_h(p0, n)
                soff += ns

            nc.sync.dma_start(d_hout.ap(), h_sl[:])

    in_maps = []
    for c in range(NCORES):
        in_maps.append({
            "wstream": wstream[c], "masks": masks_x[c],
            "xt": xT_b, "iouxstat": iouxstat[c], "iouhstat": iouhstat[c],
            "fxstat": fxstat[c], "fhstat": fhstat[c],
            "b_xi": b_xi[c], "b_iou": b_iou[c], "b_xf": b_xf[c],
            "b_fh": b_fh[c],
        })
    _split_multi_waits(nc)
    kernel._nc = nc
    kernel._in_maps = in_maps
    res = run_bass_kernel_spmd(nc, in_maps, list(range(NCORES)))
    hT = np.concatenate([res.results[c]["hout"] for c in range(NCORES)], 0)
    out = np.empty((N, MEM), np.float32)
    for node in range(N):
        out[node] = hT[:, col_of[node]]
    return out
